# revision 45
# baseline (speedup 1.0000x reference)
"""Causal GQA attention block (B=2,S=2048,D=1024,H=16,KH=4,DK=64) on 8 TRN2 cores.

Sharding: core c -> (batch b=c//4, kv-group g=c%4). Each core computes its
batch's 4 query heads (one kv head), Wq/Wk/Wv column-parallel, Wo
row-parallel; per-core partial outputs (out^T layout) are summed on host.

v2: all matmul operands and DRAM I/O in bf16 (measured end-to-end rel-err
~4.4e-3 on hardware vs the 2e-2 gate). bf16 removes the fp32r
free-dim>=256 constraint so diagonal score/PV blocks run at exact widths,
halves x^T/out^T DMA, and enables 2x/4x DVE modes on bf16 elementwise ops.
Scheduling is tuned against the instruction cost model:
  - One shared PSUM scope (mm 2 + st 4 + ot 2 = 8 banks, with
    projection/rope-swap/transpose/out-projection tiles all rotating
    through the 2-slot mm tag) so the Tile scheduler overlaps attention on
    early q-tiles with the remaining projection chunks -- scoped pools
    would alias banks and act as a phase barrier.
  - exp exclusively on ACT (its only engine); PSUM evictions on ACT;
    causal-mask multiplies + RoPE cos-multiplies on Pool; sin-multiply,
    adds, reciprocal, softmax-normalize, V^T eviction on DVE.
  - DMA spread across queues: x^T (first chunk quartered) on SP, weights
    on the ACT queue, cos/sin/Wo + K-duplication + head-shift + out^T
    stores on the Pool SWDGE queue -- transfers occupy the issuing queue
    in the cost model, so a single queue serializes.
  - ot tiles staged PSUM->SBUF with one copy so the bank frees before the
    reciprocal-broadcast-normalize chain runs; out-projection of q-tile n
    is emitted after q-tile n+1's attention at low priority; the last
    q-tile streams its out-projection per chunk to shorten the drain
    tail.

Device algorithm per core:
  1. QKV projections from x^T with RoPE fused on eviction (q stacks of
     128 partitions = 2 heads x 64 dims; kv stack = V rows 0:64, K rows
     64:128, K then duplicated to rows 0:64 so both heads of a q-stack can
     run partition-aligned score matmuls).
  2. Flash-style causal attention without max-subtraction (scores are in
     [-7.1, 7.1] for this problem, so exp is safe): S^T blocks
     [128k, <=512q] on PE, exp on ACT (scale=1/sqrt(64)), triangular mask
     multiplies on DVE only for the leading 128 columns of diagonal
     blocks, P^T @ V on PE with a ones-column-augmented V giving the
     softmax denominator for free; two heads' block streams are
     interleaved to hide exp latency.
  3. Row-parallel out-projection producing out^T [1024, 2048] partials.
"""

import sys

sys.path.insert(0, "/opt/trn_rl_repo")

import ml_dtypes
import numpy as np

import concourse.bass as bass
import concourse.bacc as bacc
import concourse.mybir as mybir
from concourse.bass_utils import run_bass_kernel_spmd
from concourse.masks import make_identity, make_upper_triangular
from concourse.tile import TileContext

F32 = mybir.dt.float32
BF16 = mybir.dt.bfloat16
EXP = mybir.ActivationFunctionType.Exp
MULT = mybir.AluOpType.mult

B, S, D = 2, 2048, 1024
H, KH, DK = 16, 4, 64
REP = H // KH  # query heads per kv head / per core
GDIM = REP * DK  # 256 query-proj columns per core
HALF = DK // 2  # 32
SCALE = 1.0 / np.sqrt(DK)

QT = 512  # q-tile (free dim of score matmuls)
KB = 128  # k-block (partition dim of score blocks)
NQT = S // QT  # 4
NKB = S // KB  # 16
ND = D // 128  # 8 contraction tiles for projections


def build_nc() -> bass.Bass:
    nc = bacc.Bacc("TRN2", target_bir_lowering=False, debug=False)

    xt_d = nc.declare_dram_parameter("xt", [D, S], BF16, isOutput=False)
    wq0_d = nc.declare_dram_parameter("wq0", [128, ND, 128], BF16, isOutput=False)
    wq1_d = nc.declare_dram_parameter("wq1", [128, ND, 128], BF16, isOutput=False)
    wvk_d = nc.declare_dram_parameter("wvk", [128, ND, 128], BF16, isOutput=False)
    wo_d = nc.declare_dram_parameter("wo", [128, 2, D], BF16, isOutput=False)
    cos_d = nc.declare_dram_parameter("cosq", [128, S], BF16, isOutput=False)
    sin_d = nc.declare_dram_parameter("sinq", [128, S], F32, isOutput=False)
    psw_d = nc.declare_dram_parameter("pswap", [128, 128], BF16, isOutput=False)
    out_d = nc.declare_dram_parameter("outT", [D, S], BF16, isOutput=True)

    with TileContext(nc) as tc:
        with tc.tile_pool(name="persist", bufs=1) as pp:
            # ---- persistent SBUF state ----
            w_sbs = {}
            for name, d in (("wq0", wq0_d), ("wq1", wq1_d), ("wvk", wvk_d)):
                w_sbs[name] = pp.tile([128, ND, 128], BF16, tag=name, name=name)
            wq0_sb, wq1_sb, wvk_sb = (w_sbs[n] for n in ("wq0", "wq1", "wvk"))
            w_dram = {"wq0": wq0_d, "wq1": wq1_d, "wvk": wvk_d}

            def load_w(name):
                nc.sync.dma_start(w_sbs[name][:], w_dram[name][:])
            load_w("wq0")
            ident = pp.tile([64, 64], BF16, tag="ident")
            make_identity(nc, ident[:])
            tril = pp.tile([128, 128], BF16, tag="tril")
            # tril[k, q] = 1 where k <= q else 0
            make_upper_triangular(nc, tril[:], val=1.0, diag=True)

            qt0_sb = pp.tile([128, S], BF16, tag="qt0")  # heads 0,1 (roped Q^T)
            qt1_sb = pp.tile([128, S], BF16, tag="qt1")  # heads 2,3
            kt2_sb = pp.tile([128, S], BF16, tag="kt2")  # roped K^T, 0:64==64:128
            nc.vector.memset(kt2_sb[0:64, :], 0.0)
            vt_sb = pp.tile([64, S], BF16, tag="vt")     # V^T (un-roped)
            vaug_sb = pp.tile([128, NKB, 65], BF16, tag="vaug")
            nc.vector.memset(vaug_sb[:, :, 64], 1.0)
            at_sb = [pp.tile([128, S], BF16, tag=f"at{p}", name=f"at{p}")
                     for p in range(2)]

            # ---- single scope: projections + RoPE + attention ----
            # All PSUM pools coexist (mm 2 + st 4 + ot 2 = 8 banks) so the
            # scheduler can overlap attention on early q-tiles with the
            # remaining projection chunks instead of hitting a bank-aliasing
            # barrier between phases.
            with tc.tile_pool(name="mm_ps", bufs=2, space="PSUM") as mm_ps, \
                 tc.tile_pool(name="st_ps", bufs=2, space="PSUM") as st_ps, \
                 tc.tile_pool(name="ot_ps", bufs=2, space="PSUM") as ot_ps, \
                 tc.tile_pool(name="swb_sb", bufs=3) as swb_sb, \
                 tc.tile_pool(name="attn_sb", bufs=3) as asb, \
                 tc.tile_pool(name="small_sb", bufs=3) as ssb, \
                 tc.tile_pool(name="xt_pool", bufs=1) as xt_pool:
                psw_sb = pp.tile([128, 128], BF16, tag="psw")
                cos_sb = pp.tile([128, S], BF16, tag="cos")
                sin_sb = pp.tile([128, S], F32, tag="sin")
                xt_sb = xt_pool.tile([128, ND, S], BF16, tag="xt")
                xt_r = xt_d.rearrange("(t p) s -> p t s", p=128)
                wo_sb = pp.tile([128, 2, D], BF16, tag="wo")
                for c in range(NQT):
                    cs = slice(c * QT, (c + 1) * QT)
                    if c == 0:
                        # quarters so the first projection chain streams
                        # behind the arriving contraction tiles; weights ride
                        # the ACT queue, tables the Pool queue, so nothing
                        # big queues ahead of x^T on SP
                        for t0 in range(0, ND, 2):
                            nc.sync.dma_start(xt_sb[:, t0:t0 + 2, cs],
                                              xt_r[:, t0:t0 + 2, cs])
                        nc.scalar.dma_start(wvk_sb[:], wvk_d[:])
                        nc.scalar.dma_start(w_sbs["wq1"][:], wq1_d[:])
                        nc.scalar.dma_start(psw_sb[:], psw_d[:])
                        nc.gpsimd.dma_start(cos_sb[:], cos_d[:])
                        nc.gpsimd.dma_start(sin_sb[:], sin_d[:])
                    else:
                        nc.sync.dma_start(xt_sb[:, :, cs], xt_r[:, :, cs])
                    if c == 1:
                        nc.gpsimd.dma_start(wo_sb[:], wo_d[:])

                def project(w_sb, c):
                    ps = mm_ps.tile([128, QT], F32, tag="mm", name="proj")
                    for t in range(ND):
                        nc.tensor.matmul(
                            ps[:],
                            w_sb[:, t, :],
                            xt_sb[:, t, c * QT:(c + 1) * QT],
                            start=(t == 0), stop=(t == ND - 1),
                        )
                    return ps

                def rope_chunk(dst, cs, lo, hi):
                    """In-place rope of dst[lo:hi, cs]. The rotate-half swap
                    (with sign) runs on PE as a +-1 block-permutation matmul
                    over all 128 partitions (out must start at partition 0);
                    the elementwise ops then touch only rows lo:hi."""
                    sl = dst[lo:hi, cs]
                    swp = mm_ps.tile([128, QT], F32, tag="mm", name="swp")
                    nc.tensor.matmul(swp[:], psw_sb[:], dst[:, cs],
                                     start=True, stop=True)
                    swb = swb_sb.tile([128, QT], BF16, tag="swb", name="swb")
                    nc.gpsimd.tensor_tensor(sl, sl, cos_sb[lo:hi, cs], MULT)
                    nc.vector.tensor_tensor(swb[lo:hi, :], swp[lo:hi, :],
                                            sin_sb[lo:hi, cs], MULT)
                    nc.vector.tensor_add(sl, sl, swb[lo:hi, :])

                for c in range(NQT):
                    cs = slice(c * QT, (c + 1) * QT)
                    ps0 = project(wq0_sb, c)
                    ps1 = project(wq1_sb, c)
                    nc.scalar.copy(qt0_sb[:, cs], ps0[:])
                    rope_chunk(qt0_sb, cs, 0, 128)
                    nc.scalar.copy(qt1_sb[:, cs], ps1[:])
                    rope_chunk(qt1_sb, cs, 0, 128)
                    ps = project(wvk_sb, c)  # rows 0:64 = V, rows 64:128 = K
                    nc.scalar.copy(kt2_sb[64:128, cs], ps[64:128])
                    nc.scalar.copy(vt_sb[:, cs], ps[0:64])
                    rope_chunk(kt2_sb, cs, 64, 128)
                    # duplicate roped K to rows 0:64 (partition shift via DMA)
                    nc.gpsimd.dma_start(kt2_sb[0:64, cs], kt2_sb[64:128, cs])
                    # V_aug for this chunk's k-blocks via PE transpose, so
                    # q-tile c's attention has its V tiles as early as possible
                    for kt in range(4 * c, 4 * c + 4):
                        tp = mm_ps.tile([128, 64], BF16, tag="mm", name="tp")
                        nc.tensor.transpose(
                            tp[:], vt_sb[:, kt * 128:(kt + 1) * 128], ident[:]
                        )
                        nc.vector.tensor_copy(vaug_sb[:, kt, 0:64], tp[:])

                # ---- attention + out-projection ----
                out_r = out_d.rearrange("(t p) s -> p t s", p=128)

                def outproj_pieces(qt, streaming=False):
                    # out^T chunks for q-tile qt, as 4 closures woven into the
                    # next q-tile's pair loop so their PE matmuls fill exp-wait
                    # gaps instead of starving ACT in a burst at the q-tile
                    # boundary. The at1 half (heads 2,3) finishes normalizing
                    # first, so it runs as the accumulation start. The last
                    # q-tile streams per-chunk with evictions alternating
                    # ACT/DVE to shorten the drain tail.
                    def piece(dc0):
                        ob = asb.tile([128, 2, QT], BF16, tag="ob", name="ob")
                        for i in range(2):
                            dc = dc0 + i
                            op = mm_ps.tile([128, QT], F32, tag="mm", name="op")
                            for p in (1, 0):
                                nc.tensor.matmul(
                                    op[:],
                                    wo_sb[:, p, dc * 128:(dc + 1) * 128],
                                    at_sb[p][:, qt * QT:(qt + 1) * QT],
                                    start=(p == 1), stop=(p == 0),
                                )
                            if streaming and dc % 2 == 0:
                                nc.scalar.copy(ob[:, i, :], op[:])
                            else:
                                nc.vector.tensor_copy(ob[:, i, :], op[:])
                            if streaming:
                                nc.gpsimd.dma_start(
                                    out_r[:, dc, qt * QT:(qt + 1) * QT],
                                    ob[:, i, :])
                        if not streaming:
                            nc.gpsimd.dma_start(
                                out_r[:, dc0:dc0 + 2, qt * QT:(qt + 1) * QT],
                                ob[:])
                    return [lambda dc0=dc0: piece(dc0)
                            for dc0 in range(0, ND, 2)]

                for qt in range(NQT):
                    nblk = 4 * qt + 4
                    for hgrp in ((1, 3), (2, 0)):
                        # interleave two heads' block-pairs so each head's
                        # exp latency hides behind the other head's matmuls
                        ots = {}
                        for h in hgrp:
                            ots[h] = ot_ps.tile([65, QT], F32, tag="ot",
                                                name="ot")
                        for kb0 in range(0, nblk, 2):
                          for h in hgrp:
                            qsrc = qt0_sb if h < 2 else qt1_sb
                            qrow = 64 * (h % 2)
                            st = st_ps.tile([128, 2 * QT], F32, tag="st", name="st")
                            pt = asb.tile([128, 2 * QT], BF16, tag="pt", name="pt")
                            cols = []  # (kb, moff, n, col)
                            col = 0
                            for kb in (kb0, kb0 + 1):
                                moff = max(0, (kb - 4 * qt) * 128)
                                n = QT - moff
                                if col % QT + n > QT:  # stay inside a psum bank
                                    col = (col // QT + 1) * QT
                                nc.tensor.matmul(
                                    st[:, col:col + n],
                                    kt2_sb[qrow:qrow + 64,
                                           kb * 128:(kb + 1) * 128],
                                    qsrc[qrow:qrow + 64,
                                         qt * QT + moff:(qt + 1) * QT],
                                    start=True, stop=True,
                                )
                                cols.append((kb, moff, n, col))
                                col += n
                            tot = cols[-1][3] + cols[-1][2]
                            nc.scalar.activation(pt[:, :tot], st[:, :tot], EXP,
                                                 scale=float(SCALE))
                            ot = ots[h]
                            for kb, moff, n, col in cols:
                                if kb >= 4 * qt:  # diagonal block: mask the
                                    # leading 128 columns (q in [moff,moff+128))
                                    nc.gpsimd.tensor_tensor(
                                        pt[:, col:col + 128],
                                        pt[:, col:col + 128],
                                        tril[:], MULT)
                                nc.tensor.matmul(
                                    ot[:, moff:QT],
                                    vaug_sb[:, kb, :],
                                    pt[:, col:col + n],
                                    start=(kb == 0), stop=(kb == nblk - 1),
                                )
                        for h in hgrp:
                            qrow = 64 * (h % 2)
                            ot = ots[h]
                            if qt < NQT - 1:
                                # stage PSUM->SBUF with one copy so the ot
                                # bank frees for the next head group without
                                # waiting out the whole normalize chain; the
                                # last q-tile keeps the direct path to not
                                # lengthen the drain tail
                                otc = asb.tile([65, QT], F32, tag="otc",
                                               name="otc")
                                nc.vector.tensor_copy(otc[:], ot[:])
                                ot = otc
                            lrec = ssb.tile([1, QT], F32, tag="lrec",
                                            name="lrec")
                            nc.vector.reciprocal(lrec[:], ot[64:65, :])
                            lrecb = ssb.tile([64, QT], F32, tag="lrecb",
                                             name="lrecb")
                            nc.gpsimd.partition_broadcast(lrecb[:], lrec[:])
                            at = at_sb[h // 2]
                            if qrow == 0:
                                nc.vector.tensor_tensor(
                                    at[0:64, qt * QT:(qt + 1) * QT],
                                    ot[0:64, :], lrecb[:], MULT)
                            else:
                                atmp = ssb.tile([64, QT], BF16, tag="atmp",
                                                name="atmp")
                                nc.vector.tensor_tensor(atmp[:], ot[0:64, :],
                                                        lrecb[:], MULT)
                                nc.gpsimd.dma_start(
                                    at[64:128, qt * QT:(qt + 1) * QT],
                                    atmp[:])
                        pass
                    if qt > 0:
                        # deprioritized: fill PE idle slots, never outrank the
                        # live q-tile's score/PV stream
                        with tc.high_priority(offset=-500000):
                            for p_ in outproj_pieces(qt - 1):
                                p_()
                for p_ in outproj_pieces(NQT - 1, streaming=True):
                    p_()
    nc.compile()
    return nc


_NC_CACHE = None
_last_in_maps = None


def _get_nc():
    global _NC_CACHE
    if _NC_CACHE is None:
        _NC_CACHE = build_nc()
    return _NC_CACHE


def _rope_tables():
    theta = 10000.0 ** (-(np.arange(HALF, dtype=np.float64) / HALF))
    pos = np.arange(S, dtype=np.float64)
    freqs = pos[:, None] * theta[None, :]  # [S, 32]
    cos1 = np.cos(freqs).T.astype(np.float32)  # [32, S]
    sin1 = np.sin(freqs).T.astype(np.float32)
    cosq = np.tile(cos1, (4, 1))  # [128, S]
    sinq = np.tile(sin1, (4, 1))  # [128, S] (sign lives in pswap)
    return (np.ascontiguousarray(cosq.astype(ml_dtypes.bfloat16)),
            np.ascontiguousarray(sinq))


def _pswap():
    """P[k, m]: swp[m] = sum_k P[k, m] q[k] = rotate-half with sign, per
    64-row block: swp[0:32] = -q[32:64], swp[32:64] = +q[0:32]."""
    P = np.zeros((128, 128), dtype=np.float32)
    for b in (0, 64):
        for m in range(32):
            P[b + 32 + m, b + m] = -1.0
            P[b + m, b + 32 + m] = 1.0
    return np.ascontiguousarray(P.astype(ml_dtypes.bfloat16))


def _pack_w(w):
    """[D, M] weight slice -> [128, D//128, M] bf16 (partition-major)."""
    dd, m = w.shape
    t = dd // 128
    return np.ascontiguousarray(
        w.reshape(t, 128, m).transpose(1, 0, 2).astype(ml_dtypes.bfloat16))


def make_in_maps(x, Wq, Wk, Wv, Wo):
    cosq, sinq = _rope_tables()
    xts = [np.ascontiguousarray(x[b].T.astype(ml_dtypes.bfloat16))
           for b in range(B)]
    wslices = {}
    in_maps = []
    for c in range(8):
        b, g = divmod(c, 4)
        if g not in wslices:
            wslices[g] = {
                "wq0": _pack_w(Wq[:, g * GDIM:g * GDIM + 128]),
                "wq1": _pack_w(Wq[:, g * GDIM + 128:(g + 1) * GDIM]),
                "wvk": _pack_w(
                    np.concatenate([Wv[:, g * DK:(g + 1) * DK],
                                    Wk[:, g * DK:(g + 1) * DK]], axis=1)),
                "wo": _pack_w(Wo[g * GDIM:(g + 1) * GDIM, :]),
            }
        in_maps.append({
            "xt": xts[b],
            **wslices[g],
            "cosq": cosq,
            "sinq": sinq,
            "pswap": _pswap(),
        })
    return in_maps


def kernel(x, mask, Wq, bq, Wk, bk, Wv, bv, Wo, bo):
    x = np.asarray(x, dtype=np.float32)
    mask = np.asarray(mask)
    Wq, Wk, Wv, Wo = (np.asarray(w, dtype=np.float32) for w in (Wq, Wk, Wv, Wo))
    bq, bk, bv, bo = (np.asarray(b, dtype=np.float32) for b in (bq, bk, bv, bo))

    assert np.array_equal(
        np.asarray(mask[0, 0]), np.tril(np.ones((S, S), mask.dtype))
    ), "kernel specialized for the causal mask"
    assert not bq.any() and not bk.any(), (
        "nonzero bq/bk not supported (cannot be folded outside RoPE)"
    )

    global _last_in_maps
    in_maps = make_in_maps(x, Wq, Wk, Wv, Wo)
    _last_in_maps = in_maps
    res = run_bass_kernel_spmd(_get_nc(), in_maps, list(range(8)))
    out = np.zeros((B, S, D), dtype=np.float32)
    for c in range(8):
        out[c // 4] += res.results[c]["outT"].astype(np.float32).T
    # host-side fold of the (structurally zero) v/out biases:
    # rows of softmax(P) sum to 1, so P @ (V + 1 bv^T) @ Wo + bo
    #   = P@V@Wo + sum_g bv_g_expanded @ Wo_g + bo
    corr = bo.astype(np.float64).copy()
    if bv.any():
        for g in range(KH):
            bv_full = np.tile(bv[g * DK:(g + 1) * DK], REP)  # per query head
            corr = corr + bv_full.astype(np.float64) @ Wo[g * GDIM:(g + 1) * GDIM]
    if corr.any():
        out = out + corr[None, None, :].astype(np.float32)
    return out


# revision 46
# speedup vs baseline: 1.0174x; 1.0174x over previous
"""Causal GQA attention block (B=2,S=2048,D=1024,H=16,KH=4,DK=64) on 8 TRN2 cores.

Sharding: core c -> (batch b=c//4, kv-group g=c%4). Each core computes its
batch's 4 query heads (one kv head), Wq/Wk/Wv column-parallel, Wo
row-parallel; per-core partial outputs (out^T layout) are summed on host.

v2: all matmul operands and DRAM I/O in bf16 (measured end-to-end rel-err
~4.4e-3 on hardware vs the 2e-2 gate). bf16 removes the fp32r
free-dim>=256 constraint so diagonal score/PV blocks run at exact widths,
halves x^T/out^T DMA, and enables 2x/4x DVE modes on bf16 elementwise ops.
Scheduling is tuned against the instruction cost model:
  - One shared PSUM scope (mm 2 + st 4 + ot 2 = 8 banks, with
    projection/rope-swap/transpose/out-projection tiles all rotating
    through the 2-slot mm tag) so the Tile scheduler overlaps attention on
    early q-tiles with the remaining projection chunks -- scoped pools
    would alias banks and act as a phase barrier.
  - exp exclusively on ACT (its only engine); PSUM evictions on ACT;
    causal-mask multiplies + RoPE cos-multiplies on Pool; sin-multiply,
    adds, reciprocal, softmax-normalize, V^T eviction on DVE.
  - DMA spread across queues: x^T (first chunk quartered) on SP, weights
    on the ACT queue, cos/sin/Wo + K-duplication + head-shift + out^T
    stores on the Pool SWDGE queue -- transfers occupy the issuing queue
    in the cost model, so a single queue serializes.
  - ot tiles staged PSUM->SBUF with one copy so the bank frees before the
    reciprocal-broadcast-normalize chain runs; out-projection of q-tile n
    is emitted after q-tile n+1's attention at low priority; the last
    q-tile streams its out-projection per chunk to shorten the drain
    tail.

Device algorithm per core:
  1. QKV projections from x^T with RoPE fused on eviction (q stacks of
     128 partitions = 2 heads x 64 dims; kv stack = V rows 0:64, K rows
     64:128, K then duplicated to rows 0:64 so both heads of a q-stack can
     run partition-aligned score matmuls).
  2. Flash-style causal attention without max-subtraction (scores are in
     [-7.1, 7.1] for this problem, so exp is safe): S^T blocks
     [128k, <=512q] on PE, exp on ACT (scale=1/sqrt(64)), triangular mask
     multiplies on DVE only for the leading 128 columns of diagonal
     blocks, P^T @ V on PE with a ones-column-augmented V giving the
     softmax denominator for free; two heads' block streams are
     interleaved to hide exp latency.
  3. Row-parallel out-projection producing out^T [1024, 2048] partials.
"""

import sys

sys.path.insert(0, "/opt/trn_rl_repo")

import ml_dtypes
import numpy as np

import concourse.bass as bass
import concourse.bacc as bacc
import concourse.mybir as mybir
from concourse.bass_utils import run_bass_kernel_spmd
from concourse.masks import make_identity, make_upper_triangular
from concourse.tile import TileContext

F32 = mybir.dt.float32
BF16 = mybir.dt.bfloat16
EXP = mybir.ActivationFunctionType.Exp
MULT = mybir.AluOpType.mult

B, S, D = 2, 2048, 1024
H, KH, DK = 16, 4, 64
REP = H // KH  # query heads per kv head / per core
GDIM = REP * DK  # 256 query-proj columns per core
HALF = DK // 2  # 32
SCALE = 1.0 / np.sqrt(DK)

QT = 512  # q-tile (free dim of score matmuls)
KB = 128  # k-block (partition dim of score blocks)
NQT = S // QT  # 4
NKB = S // KB  # 16
ND = D // 128  # 8 contraction tiles for projections


def build_nc() -> bass.Bass:
    nc = bacc.Bacc("TRN2", target_bir_lowering=False, debug=False)

    xt_d = nc.declare_dram_parameter("xt", [D, S], BF16, isOutput=False)
    wq0_d = nc.declare_dram_parameter("wq0", [128, ND, 128], BF16, isOutput=False)
    wq1_d = nc.declare_dram_parameter("wq1", [128, ND, 128], BF16, isOutput=False)
    wvk_d = nc.declare_dram_parameter("wvk", [128, ND, 128], BF16, isOutput=False)
    wo_d = nc.declare_dram_parameter("wo", [128, 2, D], BF16, isOutput=False)
    cos_d = nc.declare_dram_parameter("cosq", [128, S], BF16, isOutput=False)
    sin_d = nc.declare_dram_parameter("sinq", [128, S], F32, isOutput=False)
    psw_d = nc.declare_dram_parameter("pswap", [128, 128], BF16, isOutput=False)
    out_d = nc.declare_dram_parameter("outT", [D, S], BF16, isOutput=True)

    with TileContext(nc) as tc:
        with tc.tile_pool(name="persist", bufs=1) as pp:
            # ---- persistent SBUF state ----
            w_sbs = {}
            for name, d in (("wq0", wq0_d), ("wq1", wq1_d), ("wvk", wvk_d)):
                w_sbs[name] = pp.tile([128, ND, 128], BF16, tag=name, name=name)
            wq0_sb, wq1_sb, wvk_sb = (w_sbs[n] for n in ("wq0", "wq1", "wvk"))
            w_dram = {"wq0": wq0_d, "wq1": wq1_d, "wvk": wvk_d}

            def load_w(name):
                nc.sync.dma_start(w_sbs[name][:], w_dram[name][:])
            load_w("wq0")
            ident = pp.tile([64, 64], BF16, tag="ident")
            make_identity(nc, ident[:])
            tril = pp.tile([128, 128], BF16, tag="tril")
            # tril[k, q] = 1 where k <= q else 0
            make_upper_triangular(nc, tril[:], val=1.0, diag=True)

            qt0_sb = pp.tile([128, S], BF16, tag="qt0")  # heads 0,1 (roped Q^T)
            qt1_sb = pp.tile([128, S], BF16, tag="qt1")  # heads 2,3
            kt2_sb = pp.tile([128, S], BF16, tag="kt2")  # roped K^T, 0:64==64:128
            nc.vector.memset(kt2_sb[0:64, :], 0.0)
            vt_sb = pp.tile([64, S], BF16, tag="vt")     # V^T (un-roped)
            vaug_sb = pp.tile([128, NKB, 65], BF16, tag="vaug")
            nc.vector.memset(vaug_sb[:, :, 64], 1.0)
            at_sb = [pp.tile([128, S], BF16, tag=f"at{p}", name=f"at{p}")
                     for p in range(2)]

            # ---- single scope: projections + RoPE + attention ----
            # All PSUM pools coexist (mm 2 + st 4 + ot 2 = 8 banks) so the
            # scheduler can overlap attention on early q-tiles with the
            # remaining projection chunks instead of hitting a bank-aliasing
            # barrier between phases.
            with tc.tile_pool(name="mm_ps", bufs=2, space="PSUM") as mm_ps, \
                 tc.tile_pool(name="st_ps", bufs=2, space="PSUM") as st_ps, \
                 tc.tile_pool(name="ot_ps", bufs=2, space="PSUM") as ot_ps, \
                 tc.tile_pool(name="swb_sb", bufs=3) as swb_sb, \
                 tc.tile_pool(name="attn_sb", bufs=3) as asb, \
                 tc.tile_pool(name="small_sb", bufs=3) as ssb, \
                 tc.tile_pool(name="xt_pool", bufs=1) as xt_pool:
                psw_sb = pp.tile([128, 128], BF16, tag="psw")
                cos_sb = pp.tile([128, S], BF16, tag="cos")
                sin_sb = pp.tile([128, S], F32, tag="sin")
                xt_sb = xt_pool.tile([128, ND, S], BF16, tag="xt")
                xt_r = xt_d.rearrange("(t p) s -> p t s", p=128)
                wo_sb = pp.tile([128, 2, D], BF16, tag="wo")
                for c in range(NQT):
                    cs = slice(c * QT, (c + 1) * QT)
                    if c == 0:
                        # quarters so the first projection chain streams
                        # behind the arriving contraction tiles; weights ride
                        # the ACT queue, tables the Pool queue, so nothing
                        # big queues ahead of x^T on SP
                        for t0 in range(0, ND, 2):
                            nc.sync.dma_start(xt_sb[:, t0:t0 + 2, cs],
                                              xt_r[:, t0:t0 + 2, cs])
                        nc.scalar.dma_start(wvk_sb[:], wvk_d[:])
                        nc.scalar.dma_start(w_sbs["wq1"][:], wq1_d[:])
                        nc.scalar.dma_start(psw_sb[:], psw_d[:])
                        nc.gpsimd.dma_start(cos_sb[:], cos_d[:])
                        nc.gpsimd.dma_start(sin_sb[:], sin_d[:])
                    else:
                        nc.sync.dma_start(xt_sb[:, :, cs], xt_r[:, :, cs])
                    if c == 1:
                        nc.gpsimd.dma_start(wo_sb[:], wo_d[:])

                def project(w_sb, c):
                    ps = mm_ps.tile([128, QT], F32, tag="mm", name="proj")
                    for t in range(ND):
                        nc.tensor.matmul(
                            ps[:],
                            w_sb[:, t, :],
                            xt_sb[:, t, c * QT:(c + 1) * QT],
                            start=(t == 0), stop=(t == ND - 1),
                        )
                    return ps

                def rope_chunk(dst, cs, lo, hi):
                    """In-place rope of dst[lo:hi, cs]. The rotate-half swap
                    (with sign) runs on PE as a +-1 block-permutation matmul
                    over all 128 partitions (out must start at partition 0);
                    the elementwise ops then touch only rows lo:hi."""
                    sl = dst[lo:hi, cs]
                    swp = mm_ps.tile([128, QT], F32, tag="mm", name="swp")
                    nc.tensor.matmul(swp[:], psw_sb[:], dst[:, cs],
                                     start=True, stop=True)
                    swb = swb_sb.tile([128, QT], BF16, tag="swb", name="swb")
                    nc.gpsimd.tensor_tensor(sl, sl, cos_sb[lo:hi, cs], MULT)
                    nc.vector.tensor_tensor(swb[lo:hi, :], swp[lo:hi, :],
                                            sin_sb[lo:hi, cs], MULT)
                    nc.vector.tensor_add(sl, sl, swb[lo:hi, :])

                for c in range(NQT):
                    cs = slice(c * QT, (c + 1) * QT)
                    # early chunks evict on ACT (idle before exps exist);
                    # late chunks on DVE so evictions stop interleaving into
                    # the by-then-live exp stream
                    ev = nc.scalar.copy if c < 2 else nc.vector.tensor_copy
                    ps0 = project(wq0_sb, c)
                    ps1 = project(wq1_sb, c)
                    ev(qt0_sb[:, cs], ps0[:])
                    rope_chunk(qt0_sb, cs, 0, 128)
                    ev(qt1_sb[:, cs], ps1[:])
                    rope_chunk(qt1_sb, cs, 0, 128)
                    ps = project(wvk_sb, c)  # rows 0:64 = V, rows 64:128 = K
                    ev(kt2_sb[64:128, cs], ps[64:128])
                    ev(vt_sb[:, cs], ps[0:64])
                    rope_chunk(kt2_sb, cs, 64, 128)
                    # duplicate roped K to rows 0:64 (partition shift via DMA)
                    nc.gpsimd.dma_start(kt2_sb[0:64, cs], kt2_sb[64:128, cs])
                    # V_aug for this chunk's k-blocks via PE transpose, so
                    # q-tile c's attention has its V tiles as early as possible
                    for kt in range(4 * c, 4 * c + 4):
                        tp = mm_ps.tile([128, 64], BF16, tag="mm", name="tp")
                        nc.tensor.transpose(
                            tp[:], vt_sb[:, kt * 128:(kt + 1) * 128], ident[:]
                        )
                        nc.vector.tensor_copy(vaug_sb[:, kt, 0:64], tp[:])

                # ---- attention + out-projection ----
                out_r = out_d.rearrange("(t p) s -> p t s", p=128)

                def outproj_pieces(qt, streaming=False):
                    # out^T chunks for q-tile qt, as 4 closures woven into the
                    # next q-tile's pair loop so their PE matmuls fill exp-wait
                    # gaps instead of starving ACT in a burst at the q-tile
                    # boundary. The at1 half (heads 2,3) finishes normalizing
                    # first, so it runs as the accumulation start. The last
                    # q-tile streams per-chunk with evictions alternating
                    # ACT/DVE to shorten the drain tail.
                    def piece(dc0):
                        ob = asb.tile([128, 2, QT], BF16, tag="ob", name="ob")
                        for i in range(2):
                            dc = dc0 + i
                            op = mm_ps.tile([128, QT], F32, tag="mm", name="op")
                            for p in (1, 0):
                                nc.tensor.matmul(
                                    op[:],
                                    wo_sb[:, p, dc * 128:(dc + 1) * 128],
                                    at_sb[p][:, qt * QT:(qt + 1) * QT],
                                    start=(p == 1), stop=(p == 0),
                                )
                            if streaming and dc % 2 == 0:
                                nc.scalar.copy(ob[:, i, :], op[:])
                            else:
                                nc.vector.tensor_copy(ob[:, i, :], op[:])
                            if streaming:
                                nc.gpsimd.dma_start(
                                    out_r[:, dc, qt * QT:(qt + 1) * QT],
                                    ob[:, i, :])
                        if not streaming:
                            nc.gpsimd.dma_start(
                                out_r[:, dc0:dc0 + 2, qt * QT:(qt + 1) * QT],
                                ob[:])
                    return [lambda dc0=dc0: piece(dc0)
                            for dc0 in range(0, ND, 2)]

                for qt in range(NQT):
                    nblk = 4 * qt + 4
                    for hgrp in ((1, 3), (2, 0)):
                        # interleave two heads' block-pairs so each head's
                        # exp latency hides behind the other head's matmuls
                        ots = {}
                        for h in hgrp:
                            ots[h] = ot_ps.tile([65, QT], F32, tag="ot",
                                                name="ot")
                        for kb0 in range(0, nblk, 2):
                          for h in hgrp:
                            qsrc = qt0_sb if h < 2 else qt1_sb
                            qrow = 64 * (h % 2)
                            st = st_ps.tile([128, 2 * QT], F32, tag="st", name="st")
                            pt = asb.tile([128, 2 * QT], BF16, tag="pt", name="pt")
                            cols = []  # (kb, moff, n, col)
                            col = 0
                            for kb in (kb0, kb0 + 1):
                                moff = max(0, (kb - 4 * qt) * 128)
                                n = QT - moff
                                if col % QT + n > QT:  # stay inside a psum bank
                                    col = (col // QT + 1) * QT
                                nc.tensor.matmul(
                                    st[:, col:col + n],
                                    kt2_sb[qrow:qrow + 64,
                                           kb * 128:(kb + 1) * 128],
                                    qsrc[qrow:qrow + 64,
                                         qt * QT + moff:(qt + 1) * QT],
                                    start=True, stop=True,
                                )
                                cols.append((kb, moff, n, col))
                                col += n
                            tot = cols[-1][3] + cols[-1][2]
                            nc.scalar.activation(pt[:, :tot], st[:, :tot], EXP,
                                                 scale=float(SCALE))
                            ot = ots[h]
                            for kb, moff, n, col in cols:
                                if kb >= 4 * qt:  # diagonal block: mask the
                                    # leading 128 columns (q in [moff,moff+128))
                                    nc.gpsimd.tensor_tensor(
                                        pt[:, col:col + 128],
                                        pt[:, col:col + 128],
                                        tril[:], MULT)
                                nc.tensor.matmul(
                                    ot[:, moff:QT],
                                    vaug_sb[:, kb, :],
                                    pt[:, col:col + n],
                                    start=(kb == 0), stop=(kb == nblk - 1),
                                )
                        for h in hgrp:
                            qrow = 64 * (h % 2)
                            ot = ots[h]
                            if qt < NQT - 1:
                                # stage PSUM->SBUF with one copy so the ot
                                # bank frees for the next head group without
                                # waiting out the whole normalize chain; the
                                # last q-tile keeps the direct path to not
                                # lengthen the drain tail
                                otc = asb.tile([65, QT], F32, tag="otc",
                                               name="otc")
                                nc.vector.tensor_copy(otc[:], ot[:])
                                ot = otc
                            lrec = ssb.tile([1, QT], F32, tag="lrec",
                                            name="lrec")
                            nc.vector.reciprocal(lrec[:], ot[64:65, :])
                            lrecb = ssb.tile([64, QT], F32, tag="lrecb",
                                             name="lrecb")
                            nc.gpsimd.partition_broadcast(lrecb[:], lrec[:])
                            at = at_sb[h // 2]
                            if qrow == 0:
                                nc.vector.tensor_tensor(
                                    at[0:64, qt * QT:(qt + 1) * QT],
                                    ot[0:64, :], lrecb[:], MULT)
                            else:
                                atmp = ssb.tile([64, QT], BF16, tag="atmp",
                                                name="atmp")
                                nc.vector.tensor_tensor(atmp[:], ot[0:64, :],
                                                        lrecb[:], MULT)
                                nc.gpsimd.dma_start(
                                    at[64:128, qt * QT:(qt + 1) * QT],
                                    atmp[:])
                        pass
                    if qt > 0:
                        # deprioritized: fill PE idle slots, never outrank the
                        # live q-tile's score/PV stream
                        with tc.high_priority(offset=-500000):
                            for p_ in outproj_pieces(qt - 1):
                                p_()
                for p_ in outproj_pieces(NQT - 1, streaming=True):
                    p_()
    nc.compile()
    return nc


_NC_CACHE = None
_last_in_maps = None


def _get_nc():
    global _NC_CACHE
    if _NC_CACHE is None:
        _NC_CACHE = build_nc()
    return _NC_CACHE


def _rope_tables():
    theta = 10000.0 ** (-(np.arange(HALF, dtype=np.float64) / HALF))
    pos = np.arange(S, dtype=np.float64)
    freqs = pos[:, None] * theta[None, :]  # [S, 32]
    cos1 = np.cos(freqs).T.astype(np.float32)  # [32, S]
    sin1 = np.sin(freqs).T.astype(np.float32)
    cosq = np.tile(cos1, (4, 1))  # [128, S]
    sinq = np.tile(sin1, (4, 1))  # [128, S] (sign lives in pswap)
    return (np.ascontiguousarray(cosq.astype(ml_dtypes.bfloat16)),
            np.ascontiguousarray(sinq))


def _pswap():
    """P[k, m]: swp[m] = sum_k P[k, m] q[k] = rotate-half with sign, per
    64-row block: swp[0:32] = -q[32:64], swp[32:64] = +q[0:32]."""
    P = np.zeros((128, 128), dtype=np.float32)
    for b in (0, 64):
        for m in range(32):
            P[b + 32 + m, b + m] = -1.0
            P[b + m, b + 32 + m] = 1.0
    return np.ascontiguousarray(P.astype(ml_dtypes.bfloat16))


def _pack_w(w):
    """[D, M] weight slice -> [128, D//128, M] bf16 (partition-major)."""
    dd, m = w.shape
    t = dd // 128
    return np.ascontiguousarray(
        w.reshape(t, 128, m).transpose(1, 0, 2).astype(ml_dtypes.bfloat16))


def make_in_maps(x, Wq, Wk, Wv, Wo):
    cosq, sinq = _rope_tables()
    xts = [np.ascontiguousarray(x[b].T.astype(ml_dtypes.bfloat16))
           for b in range(B)]
    wslices = {}
    in_maps = []
    for c in range(8):
        b, g = divmod(c, 4)
        if g not in wslices:
            wslices[g] = {
                "wq0": _pack_w(Wq[:, g * GDIM:g * GDIM + 128]),
                "wq1": _pack_w(Wq[:, g * GDIM + 128:(g + 1) * GDIM]),
                "wvk": _pack_w(
                    np.concatenate([Wv[:, g * DK:(g + 1) * DK],
                                    Wk[:, g * DK:(g + 1) * DK]], axis=1)),
                "wo": _pack_w(Wo[g * GDIM:(g + 1) * GDIM, :]),
            }
        in_maps.append({
            "xt": xts[b],
            **wslices[g],
            "cosq": cosq,
            "sinq": sinq,
            "pswap": _pswap(),
        })
    return in_maps


def kernel(x, mask, Wq, bq, Wk, bk, Wv, bv, Wo, bo):
    x = np.asarray(x, dtype=np.float32)
    mask = np.asarray(mask)
    Wq, Wk, Wv, Wo = (np.asarray(w, dtype=np.float32) for w in (Wq, Wk, Wv, Wo))
    bq, bk, bv, bo = (np.asarray(b, dtype=np.float32) for b in (bq, bk, bv, bo))

    assert np.array_equal(
        np.asarray(mask[0, 0]), np.tril(np.ones((S, S), mask.dtype))
    ), "kernel specialized for the causal mask"
    assert not bq.any() and not bk.any(), (
        "nonzero bq/bk not supported (cannot be folded outside RoPE)"
    )

    global _last_in_maps
    in_maps = make_in_maps(x, Wq, Wk, Wv, Wo)
    _last_in_maps = in_maps
    res = run_bass_kernel_spmd(_get_nc(), in_maps, list(range(8)))
    out = np.zeros((B, S, D), dtype=np.float32)
    for c in range(8):
        out[c // 4] += res.results[c]["outT"].astype(np.float32).T
    # host-side fold of the (structurally zero) v/out biases:
    # rows of softmax(P) sum to 1, so P @ (V + 1 bv^T) @ Wo + bo
    #   = P@V@Wo + sum_g bv_g_expanded @ Wo_g + bo
    corr = bo.astype(np.float64).copy()
    if bv.any():
        for g in range(KH):
            bv_full = np.tile(bv[g * DK:(g + 1) * DK], REP)  # per query head
            corr = corr + bv_full.astype(np.float64) @ Wo[g * GDIM:(g + 1) * GDIM]
    if corr.any():
        out = out + corr[None, None, :].astype(np.float32)
    return out
